# revision 47
# baseline (speedup 1.0000x reference)
"""Trainium2 Bass kernel: multi-head self-attention block (B=16, N=1024, C=768, H=12).

Data-parallel over batch: 8 NeuronCores x 2 batches each, no collectives.

v4 (from v2 baseline ~325us; PE streaming floor ~290us):
  * Host prepares CONTIGUOUS dram layouts per SBUF tile (wq/wk/wv split out
    of W_qkv, x chunk-major per batch): every prologue DMA is a full-rate
    contiguous read instead of a strided slab at ~1/3 bandwidth.
  * Ramp-critical stream split across both queues (sync: wq-hp0 chunks + x0
    even chunks; gpsimd: wk-hp0 + x0 odd chunks), cc-major, so the first qk
    matmul fires early and chases the stream. Bulk follows in-queue.
  * Norm: both heads' U-psum evacuations happen before the z/recip/mul math
    (frees the flex PSUM ring early).
  * Epilogue: ALL 16 proj(b1) groups pre-accumulate cc0..4 into a rotating
    PSUM ring and evacuate partials into the y tiles (Scalar engine does the
    copies - it is idle once the last exp is done) while the final norm chain
    drains. After it, only 16 single-matmul cc5 "finals" + in-place adds +
    wide y DMAs on both queues remain.
  * b_proj applied on host (it is a free elementwise add on the output).

Dataflow per core (all-transposed activations; no on-chip transposes):
  host: xT = x_shard^T                                  [C, T]
  qT/kT(hp,b) = Wq/Wk^T-slices @ xT(b)                  [128, N]
  v'   = xT-tiles^T @ Wv  (+ ones col/head)             [N, H*(HD+1)]
  S^T  = k^T-slices^T @ q^T   (per head, K=64)          [Nk, Nq]
  E    = exp(SCALE * S^T)     (ScalarE, PSUM->SBUF)     [Nk, Nq]
  U'   = v'^T @ E  (accum over k; row HD = softmax Z)   [HD+1, Nq]
  aoT  = U'[:HD] * (1/Z broadcast)                      [C, N]
  y    = aoT-tiles^T @ W_proj                           [N, C]
"""

import sys

for _p in ("/opt/trn_rl_repo", "/opt/pypackages"):
    if _p not in sys.path:
        sys.path.append(_p)

import numpy as np

B, N, C, H = 16, 1024, 768, 12
HD = C // H            # 64
SCALE = HD ** -0.5
NCORES = 8
BL = B // NCORES       # 2 batches per core
T = BL * N             # 2048 tokens per core

COMPUTE = "bf16"       # "bf16" | "f32r"


def build_attention_nc(compute=COMPUTE, bl=BL, n=N, c=C, h=H):
    import concourse.bass as bass
    import concourse.tile as tile
    from concourse import bacc, mybir
    from contextlib import ExitStack

    hd = c // h
    t = bl * n
    scale = hd ** -0.5
    assert c % 128 == 0 and n % 512 == 0 and h % 2 == 0 and hd == 64
    CCH = c // 128      # contraction chunks over channels (6)
    NHP = h // 2        # head pairs (6)
    NQ = n // 512       # 512-wide q tiles per sequence (2)
    NKT = n // 128      # 128-wide k tiles per sequence (8)
    NTT = n // 128      # 128-wide token tiles per sequence (8)
    VW = hd + 1         # v' width per head (ones col at hd)
    PH = c // 2         # proj/v free-dim half (384), <= 1 PSUM bank
    NXH = n // 512      # 512-col x halves per batch (2)

    FP32 = mybir.dt.float32
    SD = mybir.dt.bfloat16 if compute == "bf16" else FP32  # storage dtype

    def mm(ap):
        return ap.bitcast(mybir.dt.float32r) if compute == "f32r" else ap

    nc = bacc.Bacc("TRN2", target_bir_lowering=False, debug=False,
                   num_devices=NCORES)

    # host-side PARTITION-MAJOR packed images (see make_in_maps): every DMA
    # is a straight [128, X] contiguous copy at full descriptor efficiency.
    NXG = CCH // 2      # x chunk-pair groups (3)
    xT_d = nc.dram_tensor("xT", [bl * NXG * 128, 2 * n], SD,
                          kind="ExternalInput").ap()
    wq_d = nc.dram_tensor("w_q", [128, NHP * CCH * 128], SD,
                          kind="ExternalInput").ap()      # (hp, cc, f)-major
    wk_d = nc.dram_tensor("w_k", [128, NHP * CCH * 128], SD,
                          kind="ExternalInput").ap()
    wv_d = nc.dram_tensor("w_v", [128, 2 * CCH * PH], SD,
                          kind="ExternalInput").ap()      # (half, cc, f)-major
    wq0_d = nc.dram_tensor("w_q0", [128, CCH * 128], SD,
                           kind="ExternalInput").ap()     # hp0 of wq
    wk0_d = nc.dram_tensor("w_k0", [128, CCH * 128], SD,
                           kind="ExternalInput").ap()     # hp0 of wk
    wproj_d = nc.dram_tensor("w_proj", [128, CCH * c], SD,
                             kind="ExternalInput").ap()   # (cc, f)-major
    # y ships as bf16 (halves the output-DMA drain; host upconverts).
    # Costs ~0.3% extra rounding on y only - budget is 2e-2, we run 5.4e-3.
    out_d = nc.dram_tensor("out", [t, c], SD, kind="ExternalOutput").ap()

    Exp = mybir.ActivationFunctionType.Exp
    Copy = mybir.ActivationFunctionType.Copy

    units = [(b, hp) for b in range(bl) for hp in range(NHP)]
    NU = len(units)     # 12

    with tile.TileContext(nc) as tc, ExitStack() as ctx:
        consts = ctx.enter_context(tc.tile_pool(name="consts", bufs=1))
        xp = ctx.enter_context(tc.tile_pool(name="xp", bufs=2))
        qkp = ctx.enter_context(tc.tile_pool(name="qkp", bufs=3))
        vp = ctx.enter_context(tc.tile_pool(name="vp", bufs=2))
        ep = ctx.enter_context(tc.tile_pool(name="ep", bufs=9))
        aop = ctx.enter_context(tc.tile_pool(name="aop", bufs=2))
        smp = ctx.enter_context(tc.tile_pool(name="smp", bufs=1))
        yp = ctx.enter_context(tc.tile_pool(name="yp", bufs=8))
        ps_s = ctx.enter_context(tc.tile_pool(name="ps_s", bufs=2, space="PSUM"))
        ps_f = ctx.enter_context(tc.tile_pool(name="ps_f", bufs=4, space="PSUM"))

        # ---------------- DMA prologue --------------------------------------
        # All DMAs are straight partition-major copies. Per-queue DMA rate is
        # only ~105 GB/s, so the ramp-critical stream (hp0 weights + x(b0))
        # is spread over all three DMA queues in chase order, small pieces
        # first:
        #   sync:   x0-cc0 | x0-cc1 | wqA(hp1-2) | x1g0 | x1g1
        #   gpsimd: wq0 | x0-cc2,3 | wvA(h0) | wkA(hp1-2) | wkB | x1g2
        #   scalar: wk0 | x0-cc4,5 | wvB(h1) | wqB(hp3-5) | wproj
        wq0_sb = consts.tile([128, CCH, 128], SD, tag="wq0")
        wk0_sb = consts.tile([128, CCH, 128], SD, tag="wk0")
        wq_hp0 = [wq0_sb[:, cc, :] for cc in range(CCH)]
        wk_hp0 = [wk0_sb[:, cc, :] for cc in range(CCH)]
        xg = [[xp.tile([128, 2, n], SD, tag=f"xg{g}", name=f"xg_b{b}g{g}")
               for g in range(NXG)] for b in range(bl)]
        xT_all = [[[xg[b][cc // 2][:, cc % 2, xh * 512:(xh + 1) * 512]
                    for xh in range(NXH)] for cc in range(CCH)]
                  for b in range(bl)]
        wqt = consts.tile([128, NHP, CCH, 128], SD, tag="wqt")
        wkt = consts.tile([128, NHP, CCH, 128], SD, tag="wkt")
        wvt = consts.tile([128, 2, CCH, PH], SD, tag="wvt")
        wpt = consts.tile([128, CCH, c], SD, tag="wpt")
        wproj_sb = [wpt[:, cc, :] for cc in range(CCH)]

        def wq_ap(cc, hp):
            return wqt[:, hp, cc, :]

        def wk_ap(cc, hp):
            return wkt[:, hp, cc, :]

        def wv_ap(cc, half):
            return wvt[:, half, cc, :]

        def r3(dram_ap, j):
            return dram_ap.rearrange("p (j f) -> p j f", j=j)

        def x_dma(q, b, cc0, cc1):
            g, j0 = cc0 // 2, cc0 % 2
            q.dma_start(
                out=xg[b][g][:, j0:j0 + (cc1 - cc0)],
                in_=r3(xT_d[(b * NXG + g) * 128:(b * NXG + g + 1) * 128,
                            j0 * n:(j0 + (cc1 - cc0)) * n], cc1 - cc0))

        nc.gpsimd.dma_start(out=wq0_sb, in_=r3(wq0_d[:, :], CCH))
        nc.scalar.dma_start(out=wk0_sb, in_=r3(wk0_d[:, :], CCH))
        x_dma(nc.sync, 0, 0, 1)
        x_dma(nc.sync, 0, 1, 2)
        x_dma(nc.gpsimd, 0, 3, 4)
        x_dma(nc.gpsimd, 0, 4, 5)
        x_dma(nc.scalar, 0, 5, 6)
        x_dma(nc.scalar, 0, 2, 3)

        def w_piece(q, dst, src, h0, h1):
            q.dma_start(
                out=dst[:, h0:h1],
                in_=src[:, h0 * CCH * 128:h1 * CCH * 128].rearrange(
                    "p (hp cc f) -> p hp cc f", hp=h1 - h0, cc=CCH))

        # wv in cc-range pieces so v-groups chase the stream (accumulation
        # starts at cc0 long before the tail chunks land)
        for vh, q in ((0, nc.gpsimd), (1, nc.scalar)):
            for c0, c1 in ((0, 3), (3, CCH)):
                q.dma_start(
                    out=wvt[:, vh, c0:c1],
                    in_=wv_d[:, (vh * CCH + c0) * PH:(vh * CCH + c1) * PH]
                    .rearrange("p (cc f) -> p cc f", cc=c1 - c0))
        w_piece(nc.sync, wqt, wq_d, 1, 3)
        w_piece(nc.gpsimd, wkt, wk_d, 1, 3)
        w_piece(nc.gpsimd, wkt, wk_d, 3, 6)
        w_piece(nc.scalar, wqt, wq_d, 3, 6)
        for g, q in ((0, nc.sync), (1, nc.sync), (2, nc.gpsimd)):
            nc_q = q
            nc_q.dma_start(
                out=xg[1][g],
                in_=r3(xT_d[(NXG + g) * 128:(NXG + g + 1) * 128, :], 2))
        nc.scalar.dma_start(out=wpt, in_=r3(wproj_d[:, :], CCH))

        # ---------------- building-block emitters --------------------------
        qt_all = {}   # (b, hp) -> [128, n] q^T tile (2 heads stacked)
        kt_all = {}
        v_all = [[None] * NTT for _ in range(bl)]
        e_all = {}    # (b, hp, kt, head) -> E tile
        u_ps = {}     # (b, hp) -> [head][qn] psum accumulators
        ao_all = {}   # (b, hp) -> [128, n] normalized attention output^T
        y_tiles = {}  # (b, tt) -> ([128, c] tile, halves-finished count)

        def emit_qk_group(b, hp, dst, qn):
            """Project one 512-token slice of q^T (dst=0) or k^T (dst=1)."""
            key = (b, hp)
            store = qt_all if dst == 0 else kt_all
            if key not in store:
                store[key] = qkp.tile([128, n], SD, tag=f"qk{dst}",
                                      name=f"{'qk'[dst]}t_b{b}hp{hp}")
            ps = ps_f.tile([128, 512], FP32, tag="u",
                           name=f"qkps_b{b}hp{hp}d{dst}q{qn}")
            # unit 0 accumulates in DMA-arrival order (x pieces land
            # out-of-order across the three queues); psum accumulation
            # is order-independent
            ccs = [0, 3, 5, 1, 4, 2] if (b, hp) == (0, 0) \
                else list(range(CCH))
            for j, cc in enumerate(ccs):
                if hp == 0:
                    w_ap = (wq_hp0 if dst == 0 else wk_hp0)[cc]
                else:
                    w_ap = (wq_ap if dst == 0 else wk_ap)(cc, hp)
                nc.tensor.matmul(
                    ps,
                    lhsT=mm(w_ap),
                    rhs=mm(xT_all[b][cc][qn]),
                    start=(j == 0), stop=(j == CCH - 1))
            with tc.high_priority(offset=300):
                nc.vector.tensor_copy(
                    store[key][:, qn * 512:(qn + 1) * 512], ps)

        def emit_v_group(b, tt, half):
            """One [128-token, 384-channel] slice of v' (+ones cols)."""
            if half == 0:
                vt = vp.tile([128, h * VW], SD, tag=f"v{tt}",
                             name=f"v_b{b}t{tt}")
                ones_view = vt[:, :].rearrange(
                    "p (hh w) -> p hh w", hh=h)[:, :, hd:hd + 1]
                nc.gpsimd.memset(ones_view, 1.0)
                v_all[b][tt] = vt
            vt = v_all[b][tt]
            ps = ps_f.tile([128, PH], FP32, tag="u",
                           name=f"vps_b{b}t{tt}f{half}")
            xh, tl = tt // 4, tt % 4
            for cc in range(CCH):
                nc.tensor.matmul(
                    ps,
                    lhsT=mm(xT_all[b][cc][xh][:, tl * 128:(tl + 1) * 128]),
                    rhs=mm(wv_ap(cc, half)),
                    start=(cc == 0), stop=(cc == CCH - 1))
            nheads = PH // hd
            dst = vt[:, half * nheads * VW:(half + 1) * nheads * VW].rearrange(
                "p (hh w) -> p hh w", hh=nheads)[:, :, 0:hd]
            srcv = ps[:].rearrange("p (hh w) -> p hh w", hh=nheads)
            with tc.high_priority(offset=300):
                nc.vector.tensor_copy(dst, srcv)

        def get_y_tile(b, tt):
            if (b, tt) not in y_tiles:
                y_tiles[(b, tt)] = [yp.tile([128, c], SD, tag="y",
                                            name=f"y_b{b}t{tt}"), 0]
            return y_tiles[(b, tt)]

        tail_q = [0]

        def evac_y_half(b, tt, half, ps, evac, bump=True):
            """Copy/accumulate proj psum into the y tile; DMA when complete."""
            ent = get_y_tile(b, tt)
            yt = ent[0]
            dstv = yt[:, half * PH:(half + 1) * PH]
            if evac == "scalar":
                nc.scalar.activation(dstv, ps, Copy)
            elif evac == "add":
                with tc.high_priority(offset=300):
                    nc.vector.tensor_add(dstv, ps, dstv)
            else:
                with tc.high_priority(offset=300):
                    nc.vector.tensor_copy(dstv, ps)
            if bump:
                ent[1] += 1
                if ent[1] == 2:
                    if b == 0:
                        q = nc.sync if tt % 2 == 0 else nc.gpsimd
                    else:
                        qs = [nc.sync, nc.gpsimd, nc.scalar]
                        q = qs[tail_q[0] % 3]
                        tail_q[0] += 1
                    q.dma_start(
                        out=out_d[b * n + tt * 128:b * n + (tt + 1) * 128, :],
                        in_=yt)
                    del y_tiles[(b, tt)]

        def emit_proj_group(b, tt, half, evac="vector"):
            """One [128-token, 384-channel] output-projection slice."""
            ps = ps_f.tile([128, PH], FP32, tag="u",
                           name=f"yps_b{b}t{tt}f{half}")
            for cc in range(CCH):
                nc.tensor.matmul(
                    ps,
                    lhsT=mm(ao_all[(b, cc)][:, tt * 128:(tt + 1) * 128]),
                    rhs=mm(wproj_sb[cc][:, half * PH:(half + 1) * PH]),
                    start=(cc == 0), stop=(cc == CCH - 1))
            evac_y_half(b, tt, half, ps, evac)

        s_done = set()

        def emit_S(b, hp, kt):
            """S^T matmuls + exp for both heads of one 128-key tile."""
            if (b, hp, kt) in s_done:
                return
            s_done.add((b, hp, kt))
            qb = qt_all[(b, hp)]
            kb = kt_all[(b, hp)]
            for head in range(2):
                p0 = head * 64
                sps = ps_s.tile([128, n], FP32, tag="s",
                                name=f"s_b{b}hp{hp}k{kt}h{head}")
                for qn in range(NQ):
                    nc.tensor.matmul(
                        sps[:, qn * 512:(qn + 1) * 512],
                        lhsT=mm(kb[p0:p0 + 64, kt * 128:(kt + 1) * 128]),
                        rhs=mm(qb[p0:p0 + 64, qn * 512:(qn + 1) * 512]),
                        start=True, stop=True)
                et = ep.tile([128, n], SD, tag=f"e{head}",
                             name=f"e_b{b}hp{hp}k{kt}h{head}")
                nc.scalar.activation(et, sps, Exp, scale=scale)
                e_all[(b, hp, kt, head)] = et

        def emit_U_chunk(b, hp, kts):
            """U matmuls for key-tiles `kts`; their E tiles are all ready."""
            if (b, hp) not in u_ps:
                u_ps[(b, hp)] = [[ps_f.tile([VW, 512], FP32, tag="u",
                                            name=f"u_b{b}hp{hp}h{hh}q{qn}")
                                  for qn in range(NQ)] for hh in range(2)]
            ups = u_ps[(b, hp)]
            for kt in kts:
                for head in range(2):
                    hh = 2 * hp + head
                    et = e_all.pop((b, hp, kt, head))
                    for qn in range(NQ):
                        nc.tensor.matmul(
                            ups[head][qn],
                            lhsT=mm(v_all[b][kt][:, hh * VW:hh * VW + VW]),
                            rhs=mm(et[:, qn * 512:(qn + 1) * 512]),
                            start=(kt == 0), stop=(kt == NKT - 1))

        def emit_norm_evac(b, hp):
            """Copy both heads' U psums to SBUF (frees the flex ring)."""
            ups = u_ps.pop((b, hp))
            usb = {}
            for head in (1, 0):
                usb[head] = smp.tile([VW, n], FP32, tag=f"usb{head}",
                                     name=f"usb_b{b}hp{hp}h{head}")
            # qn-major: the split-norm's first half-chain (tokens 0-511)
            # can start after the first two copies
            for qn in range(NQ):
                for head in (1, 0):
                    with tc.high_priority(offset=300):
                        nc.vector.tensor_copy(
                            usb[head][:, qn * 512:(qn + 1) * 512],
                            ups[head][qn])
            return usb

        def norm_cols(b, hp, usb, ao, c0, c1):
            """Divide one column range by Z, fill that range of aoT.

            Z -> partition 0 (DMA), broadcast to 64 partitions (gpsimd),
            reciprocal on the broadcast tile (base-partition!=0 sources
            mis-execute on hw for both the DVE op and the broadcast).
            """
            w = c1 - c0
            for head in (1, 0):
                ut = usb[head]
                z1 = smp.tile([1, n], FP32, tag=f"z1{head}", bufs=1,
                              name=f"z1_b{b}hp{hp}h{head}c{c0}")
                nc.sync.dma_start(out=z1[:, 0:w], in_=ut[hd:hd + 1, c0:c1])
                rb = smp.tile([64, n], FP32, tag=f"rb{head}",
                              name=f"rb_b{b}hp{hp}h{head}c{c0}")
                nc.gpsimd.partition_broadcast(rb[:, 0:w], z1[:, 0:w])
                nc.vector.reciprocal_approx_fast(rb[:, 0:w], rb[:, 0:w])
                if head == 0:
                    nc.vector.tensor_mul(ao[0:64, c0:c1], ut[0:hd, c0:c1],
                                         rb[:, 0:w])
                else:
                    sc = smp.tile([64, n], SD, tag="sc",
                                  name=f"sc_b{b}hp{hp}c{c0}")
                    nc.vector.tensor_mul(sc[:, 0:w], ut[0:hd, c0:c1],
                                         rb[:, 0:w])
                    nc.sync.dma_start(out=ao[64:128, c0:c1], in_=sc[:, 0:w])

        def alloc_ao(b, hp):
            ao = aop.tile([128, n], SD, tag=f"ao{hp}", name=f"ao_b{b}hp{hp}")
            ao_all[(b, hp)] = ao
            return ao

        def emit_norm_math(b, hp, usb):
            norm_cols(b, hp, usb, alloc_ao(b, hp), 0, n)

        def emit_norm(b, hp):
            emit_norm_math(b, hp, emit_norm_evac(b, hp))

        # proj(b1) groups: cc0..4 partials land in the y tiles early, a
        # single cc5 "final" + in-place add completes them after the last
        # norm. rest = the 12 non-warm groups.
        rest = [(tt, half) for tt in range(2, NTT) for half in range(2)]
        partial_done = set()

        def proj_mms(tt, half, ps, ccs, start, stop):
            for cc in ccs:
                nc.tensor.matmul(
                    ps,
                    lhsT=mm(ao_all[(1, cc)][:, tt * 128:(tt + 1) * 128]),
                    rhs=mm(wproj_sb[cc][:, half * PH:(half + 1) * PH]),
                    start=start and cc == ccs[0],
                    stop=stop and cc == ccs[-1])

        def emit_partial_rest(g):
            if g in partial_done:
                return
            partial_done.add(g)
            tt, half = rest[g]
            ps = ps_f.tile([128, PH], FP32, tag="u", name=f"ypart{g}")
            proj_mms(tt, half, ps, range(CCH - 1), True, True)
            evac_y_half(1, tt, half, ps,
                        "scalar" if g % 2 == 0 else "vector", bump=False)

        # ---------------- filler schedule ----------------------------------
        # per-unit list of thunks run between S groups of that unit
        fillers = [[] for _ in range(NU)]

        def add_qk_fillers(i, b, hp):
            for qn in range(NQ):
                for dst in range(2):
                    fillers[i].append(
                        lambda b=b, hp=hp, dst=dst, qn=qn:
                        emit_qk_group(b, hp, dst, qn))

        # unit 0: v(b0) fully (half0 first: its wv piece lands first) + qk(u1)
        for half in range(2):
            for tt in range(NTT):
                fillers[0].append(
                    lambda tt=tt, half=half: emit_v_group(0, tt, half))
        add_qk_fillers(0, *units[1])
        # units 1..4: qk(next) + v(b1) spread 4 per unit
        for i in range(1, 5):
            add_qk_fillers(i, *units[i + 1])
        vq = [(tt, half) for tt in range(NTT) for half in range(2)]
        for j, (tt, half) in enumerate(vq):
            fillers[1 + j // 4].append(
                lambda tt=tt, half=half: emit_v_group(1, tt, half))
        # units 5..10: qk(next)
        for i in range(5, 11):
            add_qk_fillers(i, *units[i + 1])
        # units 7..10: proj(b0)  (all ao(b0) ready after norm(u5) in unit 6;
        # unit 11 keeps its flex psum free for in-unit U accumulation)
        pq = [(tt, half) for tt in range(NTT) for half in range(2)]
        for j, (tt, half) in enumerate(pq):
            fillers[7 + j % 4].append(
                lambda tt=tt, half=half: emit_proj_group(0, tt, half))

        # ---------------- main schedule ------------------------------------
        # prologue: qk(unit0), qn-major to chase the x DMA stream
        b0, hp0 = units[0]
        for qn in range(NQ):
            for dst in range(2):
                emit_qk_group(b0, hp0, dst, qn)

        for i, (b, hp) in enumerate(units):
            fl = list(fillers[i])
            prev = units[i - 1] if i > 0 else None
            # S(k0) first so the Scalar engine stays fed across the boundary;
            # previous unit's U matmuls run in chunks between S groups so
            # exp never starves and every U operand is long since ready.
            emit_S(b, hp, 0)
            start_kt = 1
            if prev is not None:
                for j, (k0, k1) in enumerate(((0, 2), (2, 4), (4, 6), (6, 8))):
                    emit_U_chunk(prev[0], prev[1], range(k0, k1))
                    if j < 3:
                        emit_S(b, hp, j + 1)
                emit_norm(*prev)
                start_kt = 4
            # spread fillers across the remaining kt slots; the last unit
            # instead runs its own U matmuls in-unit at lag 4
            nslots = NKT - start_kt
            tot = len(fl)
            for kt in range(start_kt, NKT):
                emit_S(b, hp, kt)
                if i == NU - 1 and kt >= 4:
                    emit_U_chunk(b, hp, [kt - 4])
                j = kt - start_kt
                k = (tot * (j + 1)) // nslots - (tot * j) // nslots
                for _ in range(k):
                    if fl:
                        fl.pop(0)()
            # pull the last unit's first S/exp group into this unit's
            # Scalar-engine slack: exp(u11,k7) gates the whole tail chain
            if i == NU - 2:
                emit_S(*units[NU - 1], 0)

        # ---------------- epilogue -----------------------------------------
        # Last unit's U(k4..7); 4 warm groups (tt0-1) keep their cc0..4
        # partials IN the freed S-slot banks (final = one more accumulating
        # matmul + plain copy evac), the other 12 groups' partials rotate
        # the flex ring and land in the y tiles (in-place add finals), all
        # while the final norm chain drains. Whole-row y DMAs on 3 queues.
        pb, php = units[-1]
        emit_U_chunk(pb, php, range(4, NKT))

        warm = [(0, 0), (0, 1), (1, 0), (1, 1)]
        s_carve = [ps_s.tile([128, n], FP32, tag="s", name=f"scarve{j}")
                   for j in range(2)]
        warm_ps = {}
        for g, (tt, half) in enumerate(warm):
            ps = s_carve[g // 2][:, (g % 2) * 512:(g % 2) * 512 + PH]
            warm_ps[(tt, half)] = ps
            proj_mms(tt, half, ps, range(CCH - 1), True, False)
        usb_last = emit_norm_evac(pb, php)
        for g in range(len(rest)):
            emit_partial_rest(g)
        # norm in qn halves: finals for tt0-3 (tokens 0-511) fire after the
        # first half-chain, their DMAs overlapping the second half
        ao_last = alloc_ao(pb, php)
        cc5 = [CCH - 1]

        def finals(tts):
            for tt in tts:
                for half in range(2):
                    if (tt, half) in warm_ps:
                        ps = warm_ps[(tt, half)]
                        proj_mms(tt, half, ps, cc5, False, True)
                        evac_y_half(1, tt, half, ps, "scalar")
                    else:
                        ps = ps_f.tile([128, PH], FP32, tag="u",
                                       name=f"yfin{tt}_{half}")
                        proj_mms(tt, half, ps, cc5, True, True)
                        evac_y_half(1, tt, half, ps, "add")

        norm_cols(pb, php, usb_last, ao_last, 0, 512)
        finals(range(0, 4))
        norm_cols(pb, php, usb_last, ao_last, 512, n)
        finals(range(4, NTT))

    nc.compile()
    return nc


_NC_CACHE = {}


def _get_nc(compute=COMPUTE):
    if compute not in _NC_CACHE:
        _NC_CACHE[compute] = build_attention_nc(compute)
    return _NC_CACHE[compute]


def make_in_maps(x, W_qkv, W_proj, b_proj, compute=None):
    compute = compute or COMPUTE
    if compute == "bf16":
        import ml_dtypes
        sd = ml_dtypes.bfloat16
    else:
        sd = np.float32
    x = np.asarray(x, dtype=np.float32)
    W_qkv = np.asarray(W_qkv, dtype=np.float32)
    CCH, NHP, PH, NXG = C // 128, H // 2, C // 2, C // 256

    def pack(w, inner):
        # [C, X] -> partition-major [128, (outer..., inner)] image
        return np.ascontiguousarray(
            w.reshape(CCH, 128, -1, inner).transpose(1, 2, 0, 3)
            .reshape(128, -1)).astype(sd)

    wq = pack(W_qkv[:, 0:C], 128)              # (hp, cc, 128)
    wk = pack(W_qkv[:, C:2 * C], 128)
    wv = pack(W_qkv[:, 2 * C:3 * C], PH)       # (half, cc, PH)
    wp = pack(np.asarray(W_proj, dtype=np.float32), C)  # (cc, C)
    wq0 = np.ascontiguousarray(
        W_qkv[:, 0:128].reshape(CCH, 128, 128).transpose(1, 0, 2)
        .reshape(128, -1)).astype(sd)
    wk0 = np.ascontiguousarray(
        W_qkv[:, C:C + 128].reshape(CCH, 128, 128).transpose(1, 0, 2)
        .reshape(128, -1)).astype(sd)
    in_maps = []
    for i in range(NCORES):
        shard = x[i * BL:(i + 1) * BL]                      # [BL, N, C]
        # (b, g, p, j, f) image: rows (b, g, p), cols (j, f)
        xT = np.ascontiguousarray(
            shard.transpose(0, 2, 1).reshape(BL, NXG, 2, 128, N)
            .transpose(0, 1, 3, 2, 4).reshape(BL * NXG * 128, 2 * N)
        ).astype(sd)
        in_maps.append({"xT": xT, "w_q": wq, "w_k": wk, "w_v": wv,
                        "w_q0": wq0, "w_k0": wk0, "w_proj": wp})
    return in_maps


def kernel(x, W_qkv, W_proj, b_proj):
    from concourse.bass_utils import run_bass_kernel_spmd

    nc = _get_nc()
    in_maps = make_in_maps(x, W_qkv, W_proj, b_proj)
    res = run_bass_kernel_spmd(nc, in_maps, core_ids=list(range(NCORES)))
    outs = [res.results[i]["out"].reshape(BL, N, C) for i in range(NCORES)]
    y = np.concatenate(outs, axis=0).astype(np.float32)
    return y + np.asarray(b_proj, dtype=np.float32)


if __name__ == "__main__":
    nc = build_attention_nc()
    print("built ok")


# revision 48
# speedup vs baseline: 1.0063x; 1.0063x over previous
"""Trainium2 Bass kernel: multi-head self-attention block (B=16, N=1024, C=768, H=12).

Data-parallel over batch: 8 NeuronCores x 2 batches each, no collectives.

v4 (from v2 baseline ~325us; PE streaming floor ~290us):
  * Host prepares CONTIGUOUS dram layouts per SBUF tile (wq/wk/wv split out
    of W_qkv, x chunk-major per batch): every prologue DMA is a full-rate
    contiguous read instead of a strided slab at ~1/3 bandwidth.
  * Ramp-critical stream split across both queues (sync: wq-hp0 chunks + x0
    even chunks; gpsimd: wk-hp0 + x0 odd chunks), cc-major, so the first qk
    matmul fires early and chases the stream. Bulk follows in-queue.
  * Norm: both heads' U-psum evacuations happen before the z/recip/mul math
    (frees the flex PSUM ring early).
  * Epilogue: ALL 16 proj(b1) groups pre-accumulate cc0..4 into a rotating
    PSUM ring and evacuate partials into the y tiles (Scalar engine does the
    copies - it is idle once the last exp is done) while the final norm chain
    drains. After it, only 16 single-matmul cc5 "finals" + in-place adds +
    wide y DMAs on both queues remain.
  * b_proj applied on host (it is a free elementwise add on the output).

Dataflow per core (all-transposed activations; no on-chip transposes):
  host: xT = x_shard^T                                  [C, T]
  qT/kT(hp,b) = Wq/Wk^T-slices @ xT(b)                  [128, N]
  v'   = xT-tiles^T @ Wv  (+ ones col/head)             [N, H*(HD+1)]
  S^T  = k^T-slices^T @ q^T   (per head, K=64)          [Nk, Nq]
  E    = exp(SCALE * S^T)     (ScalarE, PSUM->SBUF)     [Nk, Nq]
  U'   = v'^T @ E  (accum over k; row HD = softmax Z)   [HD+1, Nq]
  aoT  = U'[:HD] * (1/Z broadcast)                      [C, N]
  y    = aoT-tiles^T @ W_proj                           [N, C]
"""

import sys

for _p in ("/opt/trn_rl_repo", "/opt/pypackages"):
    if _p not in sys.path:
        sys.path.append(_p)

import numpy as np

B, N, C, H = 16, 1024, 768, 12
HD = C // H            # 64
SCALE = HD ** -0.5
NCORES = 8
BL = B // NCORES       # 2 batches per core
T = BL * N             # 2048 tokens per core

COMPUTE = "bf16"       # "bf16" | "f32r"


def build_attention_nc(compute=COMPUTE, bl=BL, n=N, c=C, h=H):
    import concourse.bass as bass
    import concourse.tile as tile
    from concourse import bacc, mybir
    from contextlib import ExitStack

    hd = c // h
    t = bl * n
    scale = hd ** -0.5
    assert c % 128 == 0 and n % 512 == 0 and h % 2 == 0 and hd == 64
    CCH = c // 128      # contraction chunks over channels (6)
    NHP = h // 2        # head pairs (6)
    NQ = n // 512       # 512-wide q tiles per sequence (2)
    NKT = n // 128      # 128-wide k tiles per sequence (8)
    NTT = n // 128      # 128-wide token tiles per sequence (8)
    VW = hd + 1         # v' width per head (ones col at hd)
    PH = c // 2         # proj/v free-dim half (384), <= 1 PSUM bank
    NXH = n // 512      # 512-col x halves per batch (2)

    FP32 = mybir.dt.float32
    SD = mybir.dt.bfloat16 if compute == "bf16" else FP32  # storage dtype

    def mm(ap):
        return ap.bitcast(mybir.dt.float32r) if compute == "f32r" else ap

    nc = bacc.Bacc("TRN2", target_bir_lowering=False, debug=False,
                   num_devices=NCORES)

    # host-side PARTITION-MAJOR packed images (see make_in_maps): every DMA
    # is a straight [128, X] contiguous copy at full descriptor efficiency.
    NXG = CCH // 2      # x chunk-pair groups (3)
    xT_d = nc.dram_tensor("xT", [bl * NXG * 128, 2 * n], SD,
                          kind="ExternalInput").ap()
    wq_d = nc.dram_tensor("w_q", [128, NHP * CCH * 128], SD,
                          kind="ExternalInput").ap()      # (hp, cc, f)-major
    wk_d = nc.dram_tensor("w_k", [128, NHP * CCH * 128], SD,
                          kind="ExternalInput").ap()
    wv_d = nc.dram_tensor("w_v", [128, 2 * CCH * PH], SD,
                          kind="ExternalInput").ap()      # (half, cc, f)-major
    wq0_d = nc.dram_tensor("w_q0", [128, CCH * 128], SD,
                           kind="ExternalInput").ap()     # hp0 of wq
    wk0_d = nc.dram_tensor("w_k0", [128, CCH * 128], SD,
                           kind="ExternalInput").ap()     # hp0 of wk
    wproj_d = nc.dram_tensor("w_proj", [128, CCH * c], SD,
                             kind="ExternalInput").ap()   # (cc, f)-major
    # y ships as bf16 (halves the output-DMA drain; host upconverts).
    # Costs ~0.3% extra rounding on y only - budget is 2e-2, we run 5.4e-3.
    out_d = nc.dram_tensor("out", [t, c], SD, kind="ExternalOutput").ap()

    Exp = mybir.ActivationFunctionType.Exp
    Copy = mybir.ActivationFunctionType.Copy

    units = [(b, hp) for b in range(bl) for hp in range(NHP)]
    NU = len(units)     # 12

    with tile.TileContext(nc) as tc, ExitStack() as ctx:
        consts = ctx.enter_context(tc.tile_pool(name="consts", bufs=1))
        xp = ctx.enter_context(tc.tile_pool(name="xp", bufs=2))
        qkp = ctx.enter_context(tc.tile_pool(name="qkp", bufs=3))
        vp = ctx.enter_context(tc.tile_pool(name="vp", bufs=2))
        ep = ctx.enter_context(tc.tile_pool(name="ep", bufs=9))
        aop = ctx.enter_context(tc.tile_pool(name="aop", bufs=2))
        smp = ctx.enter_context(tc.tile_pool(name="smp", bufs=1))
        yp = ctx.enter_context(tc.tile_pool(name="yp", bufs=8))
        ps_s = ctx.enter_context(tc.tile_pool(name="ps_s", bufs=2, space="PSUM"))
        ps_f = ctx.enter_context(tc.tile_pool(name="ps_f", bufs=4, space="PSUM"))

        # ---------------- DMA prologue --------------------------------------
        # All DMAs are straight partition-major copies. Per-queue DMA rate is
        # only ~105 GB/s, so the ramp-critical stream (hp0 weights + x(b0))
        # is spread over all three DMA queues in chase order, small pieces
        # first:
        #   sync:   x0-cc0 | x0-cc1 | wqA(hp1-2) | x1g0 | x1g1
        #   gpsimd: wq0 | x0-cc2,3 | wvA(h0) | wkA(hp1-2) | wkB | x1g2
        #   scalar: wk0 | x0-cc4,5 | wvB(h1) | wqB(hp3-5) | wproj
        wq0_sb = consts.tile([128, CCH, 128], SD, tag="wq0")
        wk0_sb = consts.tile([128, CCH, 128], SD, tag="wk0")
        wq_hp0 = [wq0_sb[:, cc, :] for cc in range(CCH)]
        wk_hp0 = [wk0_sb[:, cc, :] for cc in range(CCH)]
        xg = [[xp.tile([128, 2, n], SD, tag=f"xg{g}", name=f"xg_b{b}g{g}")
               for g in range(NXG)] for b in range(bl)]
        xT_all = [[[xg[b][cc // 2][:, cc % 2, xh * 512:(xh + 1) * 512]
                    for xh in range(NXH)] for cc in range(CCH)]
                  for b in range(bl)]
        wqt = consts.tile([128, NHP, CCH, 128], SD, tag="wqt")
        wkt = consts.tile([128, NHP, CCH, 128], SD, tag="wkt")
        wvt = consts.tile([128, 2, CCH, PH], SD, tag="wvt")
        wpt = consts.tile([128, CCH, c], SD, tag="wpt")
        wproj_sb = [wpt[:, cc, :] for cc in range(CCH)]

        def wq_ap(cc, hp):
            return wqt[:, hp, cc, :]

        def wk_ap(cc, hp):
            return wkt[:, hp, cc, :]

        def wv_ap(cc, half):
            return wvt[:, half, cc, :]

        def r3(dram_ap, j):
            return dram_ap.rearrange("p (j f) -> p j f", j=j)

        def x_dma(q, b, cc0, cc1):
            g, j0 = cc0 // 2, cc0 % 2
            q.dma_start(
                out=xg[b][g][:, j0:j0 + (cc1 - cc0)],
                in_=r3(xT_d[(b * NXG + g) * 128:(b * NXG + g + 1) * 128,
                            j0 * n:(j0 + (cc1 - cc0)) * n], cc1 - cc0))

        nc.gpsimd.dma_start(out=wq0_sb, in_=r3(wq0_d[:, :], CCH))
        nc.scalar.dma_start(out=wk0_sb, in_=r3(wk0_d[:, :], CCH))
        x_dma(nc.sync, 0, 0, 1)
        x_dma(nc.sync, 0, 1, 2)
        x_dma(nc.sync, 0, 2, 3)
        x_dma(nc.gpsimd, 0, 3, 4)
        x_dma(nc.gpsimd, 0, 4, 5)
        x_dma(nc.scalar, 0, 5, 6)

        def w_piece(q, dst, src, h0, h1):
            q.dma_start(
                out=dst[:, h0:h1],
                in_=src[:, h0 * CCH * 128:h1 * CCH * 128].rearrange(
                    "p (hp cc f) -> p hp cc f", hp=h1 - h0, cc=CCH))

        # wv in cc-range pieces so v-groups chase the stream (accumulation
        # starts at cc0 long before the tail chunks land)
        for vh, q in ((0, nc.gpsimd), (1, nc.scalar)):
            for c0, c1 in ((0, 3), (3, CCH)):
                q.dma_start(
                    out=wvt[:, vh, c0:c1],
                    in_=wv_d[:, (vh * CCH + c0) * PH:(vh * CCH + c1) * PH]
                    .rearrange("p (cc f) -> p cc f", cc=c1 - c0))
        w_piece(nc.sync, wqt, wq_d, 1, 3)
        w_piece(nc.gpsimd, wkt, wk_d, 1, 3)
        w_piece(nc.gpsimd, wkt, wk_d, 3, 6)
        w_piece(nc.scalar, wqt, wq_d, 3, 6)
        for g, q in ((0, nc.sync), (1, nc.sync), (2, nc.gpsimd)):
            nc_q = q
            nc_q.dma_start(
                out=xg[1][g],
                in_=r3(xT_d[(NXG + g) * 128:(NXG + g + 1) * 128, :], 2))
        nc.scalar.dma_start(out=wpt, in_=r3(wproj_d[:, :], CCH))

        # ---------------- building-block emitters --------------------------
        qt_all = {}   # (b, hp) -> [128, n] q^T tile (2 heads stacked)
        kt_all = {}
        v_all = [[None] * NTT for _ in range(bl)]
        e_all = {}    # (b, hp, kt, head) -> E tile
        u_ps = {}     # (b, hp) -> [head][qn] psum accumulators
        ao_all = {}   # (b, hp) -> [128, n] normalized attention output^T
        y_tiles = {}  # (b, tt) -> ([128, c] tile, halves-finished count)

        def emit_qk_group(b, hp, dst, qn):
            """Project one 512-token slice of q^T (dst=0) or k^T (dst=1)."""
            key = (b, hp)
            store = qt_all if dst == 0 else kt_all
            if key not in store:
                store[key] = qkp.tile([128, n], SD, tag=f"qk{dst}",
                                      name=f"{'qk'[dst]}t_b{b}hp{hp}")
            ps = ps_f.tile([128, 512], FP32, tag="u",
                           name=f"qkps_b{b}hp{hp}d{dst}q{qn}")
            # unit 0 accumulates in DMA-arrival order (x pieces land
            # out-of-order across the three queues); psum accumulation
            # is order-independent
            ccs = [0, 5, 3, 4, 1, 2] if (b, hp) == (0, 0) \
                else list(range(CCH))
            for j, cc in enumerate(ccs):
                if hp == 0:
                    w_ap = (wq_hp0 if dst == 0 else wk_hp0)[cc]
                else:
                    w_ap = (wq_ap if dst == 0 else wk_ap)(cc, hp)
                nc.tensor.matmul(
                    ps,
                    lhsT=mm(w_ap),
                    rhs=mm(xT_all[b][cc][qn]),
                    start=(j == 0), stop=(j == CCH - 1))
            with tc.high_priority(offset=300):
                nc.vector.tensor_copy(
                    store[key][:, qn * 512:(qn + 1) * 512], ps)

        def emit_v_group(b, tt, half):
            """One [128-token, 384-channel] slice of v' (+ones cols)."""
            if half == 0:
                vt = vp.tile([128, h * VW], SD, tag=f"v{tt}",
                             name=f"v_b{b}t{tt}")
                ones_view = vt[:, :].rearrange(
                    "p (hh w) -> p hh w", hh=h)[:, :, hd:hd + 1]
                nc.gpsimd.memset(ones_view, 1.0)
                v_all[b][tt] = vt
            vt = v_all[b][tt]
            ps = ps_f.tile([128, PH], FP32, tag="u",
                           name=f"vps_b{b}t{tt}f{half}")
            xh, tl = tt // 4, tt % 4
            for cc in range(CCH):
                nc.tensor.matmul(
                    ps,
                    lhsT=mm(xT_all[b][cc][xh][:, tl * 128:(tl + 1) * 128]),
                    rhs=mm(wv_ap(cc, half)),
                    start=(cc == 0), stop=(cc == CCH - 1))
            nheads = PH // hd
            dst = vt[:, half * nheads * VW:(half + 1) * nheads * VW].rearrange(
                "p (hh w) -> p hh w", hh=nheads)[:, :, 0:hd]
            srcv = ps[:].rearrange("p (hh w) -> p hh w", hh=nheads)
            with tc.high_priority(offset=300):
                nc.vector.tensor_copy(dst, srcv)

        def get_y_tile(b, tt):
            if (b, tt) not in y_tiles:
                y_tiles[(b, tt)] = [yp.tile([128, c], SD, tag="y",
                                            name=f"y_b{b}t{tt}"), 0]
            return y_tiles[(b, tt)]

        tail_q = [0]

        def evac_y_half(b, tt, half, ps, evac, bump=True):
            """Copy/accumulate proj psum into the y tile; DMA when complete."""
            ent = get_y_tile(b, tt)
            yt = ent[0]
            dstv = yt[:, half * PH:(half + 1) * PH]
            if evac == "scalar":
                nc.scalar.activation(dstv, ps, Copy)
            elif evac == "add":
                with tc.high_priority(offset=300):
                    nc.vector.tensor_add(dstv, ps, dstv)
            else:
                with tc.high_priority(offset=300):
                    nc.vector.tensor_copy(dstv, ps)
            if bump:
                ent[1] += 1
                if ent[1] == 2:
                    if b == 0:
                        q = nc.sync if tt % 2 == 0 else nc.gpsimd
                    else:
                        qs = [nc.sync, nc.gpsimd, nc.scalar]
                        q = qs[tail_q[0] % 3]
                        tail_q[0] += 1
                    q.dma_start(
                        out=out_d[b * n + tt * 128:b * n + (tt + 1) * 128, :],
                        in_=yt)
                    del y_tiles[(b, tt)]

        def emit_proj_group(b, tt, half, evac="vector"):
            """One [128-token, 384-channel] output-projection slice."""
            ps = ps_f.tile([128, PH], FP32, tag="u",
                           name=f"yps_b{b}t{tt}f{half}")
            for cc in range(CCH):
                nc.tensor.matmul(
                    ps,
                    lhsT=mm(ao_all[(b, cc)][:, tt * 128:(tt + 1) * 128]),
                    rhs=mm(wproj_sb[cc][:, half * PH:(half + 1) * PH]),
                    start=(cc == 0), stop=(cc == CCH - 1))
            evac_y_half(b, tt, half, ps, evac)

        s_done = set()

        def emit_S(b, hp, kt):
            """S^T matmuls + exp for both heads of one 128-key tile."""
            if (b, hp, kt) in s_done:
                return
            s_done.add((b, hp, kt))
            qb = qt_all[(b, hp)]
            kb = kt_all[(b, hp)]
            for head in range(2):
                p0 = head * 64
                sps = ps_s.tile([128, n], FP32, tag="s",
                                name=f"s_b{b}hp{hp}k{kt}h{head}")
                for qn in range(NQ):
                    nc.tensor.matmul(
                        sps[:, qn * 512:(qn + 1) * 512],
                        lhsT=mm(kb[p0:p0 + 64, kt * 128:(kt + 1) * 128]),
                        rhs=mm(qb[p0:p0 + 64, qn * 512:(qn + 1) * 512]),
                        start=True, stop=True)
                et = ep.tile([128, n], SD, tag=f"e{head}",
                             name=f"e_b{b}hp{hp}k{kt}h{head}")
                nc.scalar.activation(et, sps, Exp, scale=scale)
                e_all[(b, hp, kt, head)] = et

        def emit_U_chunk(b, hp, kts):
            """U matmuls for key-tiles `kts`; their E tiles are all ready."""
            if (b, hp) not in u_ps:
                u_ps[(b, hp)] = [[ps_f.tile([VW, 512], FP32, tag="u",
                                            name=f"u_b{b}hp{hp}h{hh}q{qn}")
                                  for qn in range(NQ)] for hh in range(2)]
            ups = u_ps[(b, hp)]
            for kt in kts:
                for head in range(2):
                    hh = 2 * hp + head
                    et = e_all.pop((b, hp, kt, head))
                    for qn in range(NQ):
                        nc.tensor.matmul(
                            ups[head][qn],
                            lhsT=mm(v_all[b][kt][:, hh * VW:hh * VW + VW]),
                            rhs=mm(et[:, qn * 512:(qn + 1) * 512]),
                            start=(kt == 0), stop=(kt == NKT - 1))

        def emit_norm_evac(b, hp):
            """Copy both heads' U psums to SBUF (frees the flex ring)."""
            ups = u_ps.pop((b, hp))
            usb = {}
            for head in (1, 0):
                ut = smp.tile([VW, n], FP32, tag=f"usb{head}",
                              name=f"usb_b{b}hp{hp}h{head}")
                for qn in range(NQ):
                    with tc.high_priority(offset=300):
                        nc.vector.tensor_copy(
                            ut[:, qn * 512:(qn + 1) * 512], ups[head][qn])
                usb[head] = ut
            return usb

        def norm_cols(b, hp, usb, ao, c0, c1):
            """Divide one column range by Z, fill that range of aoT.

            Z -> partition 0 (DMA), broadcast to 64 partitions (gpsimd),
            reciprocal on the broadcast tile (base-partition!=0 sources
            mis-execute on hw for both the DVE op and the broadcast).
            """
            w = c1 - c0
            for head in (1, 0):
                ut = usb[head]
                z1 = smp.tile([1, n], FP32, tag=f"z1{head}", bufs=1,
                              name=f"z1_b{b}hp{hp}h{head}c{c0}")
                nc.sync.dma_start(out=z1[:, 0:w], in_=ut[hd:hd + 1, c0:c1])
                rb = smp.tile([64, n], FP32, tag=f"rb{head}",
                              name=f"rb_b{b}hp{hp}h{head}c{c0}")
                nc.gpsimd.partition_broadcast(rb[:, 0:w], z1[:, 0:w])
                nc.vector.reciprocal_approx_fast(rb[:, 0:w], rb[:, 0:w])
                if head == 0:
                    nc.vector.tensor_mul(ao[0:64, c0:c1], ut[0:hd, c0:c1],
                                         rb[:, 0:w])
                else:
                    sc = smp.tile([64, n], SD, tag="sc",
                                  name=f"sc_b{b}hp{hp}c{c0}")
                    nc.vector.tensor_mul(sc[:, 0:w], ut[0:hd, c0:c1],
                                         rb[:, 0:w])
                    nc.sync.dma_start(out=ao[64:128, c0:c1], in_=sc[:, 0:w])

        def alloc_ao(b, hp):
            ao = aop.tile([128, n], SD, tag=f"ao{hp}", name=f"ao_b{b}hp{hp}")
            ao_all[(b, hp)] = ao
            return ao

        def emit_norm_math(b, hp, usb):
            norm_cols(b, hp, usb, alloc_ao(b, hp), 0, n)

        def emit_norm(b, hp):
            emit_norm_math(b, hp, emit_norm_evac(b, hp))

        # proj(b1) groups: cc0..4 partials land in the y tiles early, a
        # single cc5 "final" + in-place add completes them after the last
        # norm. rest = the 12 non-warm groups.
        rest = [(tt, half) for tt in range(2, NTT) for half in range(2)]
        partial_done = set()

        def proj_mms(tt, half, ps, ccs, start, stop):
            for cc in ccs:
                nc.tensor.matmul(
                    ps,
                    lhsT=mm(ao_all[(1, cc)][:, tt * 128:(tt + 1) * 128]),
                    rhs=mm(wproj_sb[cc][:, half * PH:(half + 1) * PH]),
                    start=start and cc == ccs[0],
                    stop=stop and cc == ccs[-1])

        def emit_partial_rest(g):
            if g in partial_done:
                return
            partial_done.add(g)
            tt, half = rest[g]
            ps = ps_f.tile([128, PH], FP32, tag="u", name=f"ypart{g}")
            proj_mms(tt, half, ps, range(CCH - 1), True, True)
            evac_y_half(1, tt, half, ps,
                        "scalar" if g % 2 == 0 else "vector", bump=False)

        # ---------------- filler schedule ----------------------------------
        # per-unit list of thunks run between S groups of that unit
        fillers = [[] for _ in range(NU)]

        def add_qk_fillers(i, b, hp):
            for qn in range(NQ):
                for dst in range(2):
                    fillers[i].append(
                        lambda b=b, hp=hp, dst=dst, qn=qn:
                        emit_qk_group(b, hp, dst, qn))

        # unit 0: v(b0) fully (half0 first: its wv piece lands first) + qk(u1)
        for half in range(2):
            for tt in range(NTT):
                fillers[0].append(
                    lambda tt=tt, half=half: emit_v_group(0, tt, half))
        add_qk_fillers(0, *units[1])
        # units 1..4: qk(next) + v(b1) spread 4 per unit
        for i in range(1, 5):
            add_qk_fillers(i, *units[i + 1])
        vq = [(tt, half) for tt in range(NTT) for half in range(2)]
        for j, (tt, half) in enumerate(vq):
            fillers[1 + j // 4].append(
                lambda tt=tt, half=half: emit_v_group(1, tt, half))
        # units 5..10: qk(next)
        for i in range(5, 11):
            add_qk_fillers(i, *units[i + 1])
        # units 7..10: proj(b0)  (all ao(b0) ready after norm(u5) in unit 6;
        # unit 11 keeps its flex psum free for in-unit U accumulation)
        pq = [(tt, half) for tt in range(NTT) for half in range(2)]
        for j, (tt, half) in enumerate(pq):
            fillers[7 + j % 4].append(
                lambda tt=tt, half=half: emit_proj_group(0, tt, half))

        # ---------------- main schedule ------------------------------------
        # prologue: qk(unit0), qn-major to chase the x DMA stream
        b0, hp0 = units[0]
        for qn in range(NQ):
            for dst in range(2):
                emit_qk_group(b0, hp0, dst, qn)

        for i, (b, hp) in enumerate(units):
            fl = list(fillers[i])
            prev = units[i - 1] if i > 0 else None
            # S(k0) first so the Scalar engine stays fed across the boundary;
            # previous unit's U matmuls run in chunks between S groups so
            # exp never starves and every U operand is long since ready.
            emit_S(b, hp, 0)
            start_kt = 1
            if prev is not None:
                for j, (k0, k1) in enumerate(((0, 2), (2, 4), (4, 6), (6, 8))):
                    emit_U_chunk(prev[0], prev[1], range(k0, k1))
                    if j < 3:
                        emit_S(b, hp, j + 1)
                emit_norm(*prev)
                start_kt = 4
            # spread fillers across the remaining kt slots; the last unit
            # instead runs its own U matmuls in-unit at lag 4
            nslots = NKT - start_kt
            tot = len(fl)
            for kt in range(start_kt, NKT):
                emit_S(b, hp, kt)
                if i == NU - 1 and kt >= 4:
                    emit_U_chunk(b, hp, [kt - 4])
                j = kt - start_kt
                k = (tot * (j + 1)) // nslots - (tot * j) // nslots
                for _ in range(k):
                    if fl:
                        fl.pop(0)()
            # pull the last unit's first S/exp group into this unit's
            # Scalar-engine slack: exp(u11,k7) gates the whole tail chain
            if i == NU - 2:
                emit_S(*units[NU - 1], 0)

        # ---------------- epilogue -----------------------------------------
        # Last unit's U(k4..7); 4 warm groups (tt0-1) keep their cc0..4
        # partials IN the freed S-slot banks (final = one more accumulating
        # matmul + plain copy evac), the other 12 groups' partials rotate
        # the flex ring and land in the y tiles (in-place add finals), all
        # while the final norm chain drains. Whole-row y DMAs on 3 queues.
        pb, php = units[-1]
        emit_U_chunk(pb, php, range(4, NKT))

        warm = [(0, 0), (0, 1), (1, 0), (1, 1)]
        s_carve = [ps_s.tile([128, n], FP32, tag="s", name=f"scarve{j}")
                   for j in range(2)]
        warm_ps = {}
        for g, (tt, half) in enumerate(warm):
            ps = s_carve[g // 2][:, (g % 2) * 512:(g % 2) * 512 + PH]
            warm_ps[(tt, half)] = ps
            proj_mms(tt, half, ps, range(CCH - 1), True, False)
        usb_last = emit_norm_evac(pb, php)
        for g in range(len(rest)):
            emit_partial_rest(g)
        # norm in qn halves: finals for tt0-3 (tokens 0-511) fire after the
        # first half-chain, their DMAs overlapping the second half
        ao_last = alloc_ao(pb, php)
        cc5 = [CCH - 1]

        def finals(tts):
            for tt in tts:
                for half in range(2):
                    if (tt, half) in warm_ps:
                        ps = warm_ps[(tt, half)]
                        proj_mms(tt, half, ps, cc5, False, True)
                        evac_y_half(1, tt, half, ps, "scalar")
                    else:
                        ps = ps_f.tile([128, PH], FP32, tag="u",
                                       name=f"yfin{tt}_{half}")
                        proj_mms(tt, half, ps, cc5, True, True)
                        evac_y_half(1, tt, half, ps, "add")

        norm_cols(pb, php, usb_last, ao_last, 0, 512)
        finals(range(0, 4))
        norm_cols(pb, php, usb_last, ao_last, 512, n)
        finals(range(4, NTT))

    nc.compile()
    return nc


_NC_CACHE = {}


def _get_nc(compute=COMPUTE):
    if compute not in _NC_CACHE:
        _NC_CACHE[compute] = build_attention_nc(compute)
    return _NC_CACHE[compute]


def make_in_maps(x, W_qkv, W_proj, b_proj, compute=None):
    compute = compute or COMPUTE
    if compute == "bf16":
        import ml_dtypes
        sd = ml_dtypes.bfloat16
    else:
        sd = np.float32
    x = np.asarray(x, dtype=np.float32)
    W_qkv = np.asarray(W_qkv, dtype=np.float32)
    CCH, NHP, PH, NXG = C // 128, H // 2, C // 2, C // 256

    def pack(w, inner):
        # [C, X] -> partition-major [128, (outer..., inner)] image
        return np.ascontiguousarray(
            w.reshape(CCH, 128, -1, inner).transpose(1, 2, 0, 3)
            .reshape(128, -1)).astype(sd)

    wq = pack(W_qkv[:, 0:C], 128)              # (hp, cc, 128)
    wk = pack(W_qkv[:, C:2 * C], 128)
    wv = pack(W_qkv[:, 2 * C:3 * C], PH)       # (half, cc, PH)
    wp = pack(np.asarray(W_proj, dtype=np.float32), C)  # (cc, C)
    wq0 = np.ascontiguousarray(
        W_qkv[:, 0:128].reshape(CCH, 128, 128).transpose(1, 0, 2)
        .reshape(128, -1)).astype(sd)
    wk0 = np.ascontiguousarray(
        W_qkv[:, C:C + 128].reshape(CCH, 128, 128).transpose(1, 0, 2)
        .reshape(128, -1)).astype(sd)
    in_maps = []
    for i in range(NCORES):
        shard = x[i * BL:(i + 1) * BL]                      # [BL, N, C]
        # (b, g, p, j, f) image: rows (b, g, p), cols (j, f)
        xT = np.ascontiguousarray(
            shard.transpose(0, 2, 1).reshape(BL, NXG, 2, 128, N)
            .transpose(0, 1, 3, 2, 4).reshape(BL * NXG * 128, 2 * N)
        ).astype(sd)
        in_maps.append({"xT": xT, "w_q": wq, "w_k": wk, "w_v": wv,
                        "w_q0": wq0, "w_k0": wk0, "w_proj": wp})
    return in_maps


def kernel(x, W_qkv, W_proj, b_proj):
    from concourse.bass_utils import run_bass_kernel_spmd

    nc = _get_nc()
    in_maps = make_in_maps(x, W_qkv, W_proj, b_proj)
    res = run_bass_kernel_spmd(nc, in_maps, core_ids=list(range(NCORES)))
    outs = [res.results[i]["out"].reshape(BL, N, C) for i in range(NCORES)]
    y = np.concatenate(outs, axis=0).astype(np.float32)
    return y + np.asarray(b_proj, dtype=np.float32)


if __name__ == "__main__":
    nc = build_attention_nc()
    print("built ok")


# revision 49
# speedup vs baseline: 1.0107x; 1.0044x over previous
"""Trainium2 Bass kernel: multi-head self-attention block (B=16, N=1024, C=768, H=12).

Data-parallel over batch: 8 NeuronCores x 2 batches each, no collectives.

v4 (from v2 baseline ~325us; PE streaming floor ~290us):
  * Host prepares CONTIGUOUS dram layouts per SBUF tile (wq/wk/wv split out
    of W_qkv, x chunk-major per batch): every prologue DMA is a full-rate
    contiguous read instead of a strided slab at ~1/3 bandwidth.
  * Ramp-critical stream split across both queues (sync: wq-hp0 chunks + x0
    even chunks; gpsimd: wk-hp0 + x0 odd chunks), cc-major, so the first qk
    matmul fires early and chases the stream. Bulk follows in-queue.
  * Norm: both heads' U-psum evacuations happen before the z/recip/mul math
    (frees the flex PSUM ring early).
  * Epilogue: ALL 16 proj(b1) groups pre-accumulate cc0..4 into a rotating
    PSUM ring and evacuate partials into the y tiles (Scalar engine does the
    copies - it is idle once the last exp is done) while the final norm chain
    drains. After it, only 16 single-matmul cc5 "finals" + in-place adds +
    wide y DMAs on both queues remain.
  * b_proj applied on host (it is a free elementwise add on the output).

Dataflow per core (all-transposed activations; no on-chip transposes):
  host: xT = x_shard^T                                  [C, T]
  qT/kT(hp,b) = Wq/Wk^T-slices @ xT(b)                  [128, N]
  v'   = xT-tiles^T @ Wv  (+ ones col/head)             [N, H*(HD+1)]
  S^T  = k^T-slices^T @ q^T   (per head, K=64)          [Nk, Nq]
  E    = exp(SCALE * S^T)     (ScalarE, PSUM->SBUF)     [Nk, Nq]
  U'   = v'^T @ E  (accum over k; row HD = softmax Z)   [HD+1, Nq]
  aoT  = U'[:HD] * (1/Z broadcast)                      [C, N]
  y    = aoT-tiles^T @ W_proj                           [N, C]
"""

import sys

for _p in ("/opt/trn_rl_repo", "/opt/pypackages"):
    if _p not in sys.path:
        sys.path.append(_p)

import numpy as np

B, N, C, H = 16, 1024, 768, 12
HD = C // H            # 64
SCALE = HD ** -0.5
NCORES = 8
BL = B // NCORES       # 2 batches per core
T = BL * N             # 2048 tokens per core

COMPUTE = "bf16"       # "bf16" | "f32r"


def build_attention_nc(compute=COMPUTE, bl=BL, n=N, c=C, h=H):
    import concourse.bass as bass
    import concourse.tile as tile
    from concourse import bacc, mybir
    from contextlib import ExitStack

    hd = c // h
    t = bl * n
    scale = hd ** -0.5
    assert c % 128 == 0 and n % 512 == 0 and h % 2 == 0 and hd == 64
    CCH = c // 128      # contraction chunks over channels (6)
    NHP = h // 2        # head pairs (6)
    NQ = n // 512       # 512-wide q tiles per sequence (2)
    NKT = n // 128      # 128-wide k tiles per sequence (8)
    NTT = n // 128      # 128-wide token tiles per sequence (8)
    VW = hd + 1         # v' width per head (ones col at hd)
    PH = c // 2         # proj/v free-dim half (384), <= 1 PSUM bank
    NXH = n // 512      # 512-col x halves per batch (2)

    FP32 = mybir.dt.float32
    SD = mybir.dt.bfloat16 if compute == "bf16" else FP32  # storage dtype

    def mm(ap):
        return ap.bitcast(mybir.dt.float32r) if compute == "f32r" else ap

    nc = bacc.Bacc("TRN2", target_bir_lowering=False, debug=False,
                   num_devices=NCORES)

    # host-side PARTITION-MAJOR packed images (see make_in_maps): every DMA
    # is a straight [128, X] contiguous copy at full descriptor efficiency.
    NXG = CCH // 2      # x chunk-pair groups (3)
    xT_d = nc.dram_tensor("xT", [bl * NXG * 128, 2 * n], SD,
                          kind="ExternalInput").ap()
    wq_d = nc.dram_tensor("w_q", [128, NHP * CCH * 128], SD,
                          kind="ExternalInput").ap()      # (hp, cc, f)-major
    wk_d = nc.dram_tensor("w_k", [128, NHP * CCH * 128], SD,
                          kind="ExternalInput").ap()
    wv_d = nc.dram_tensor("w_v", [128, 2 * CCH * PH], SD,
                          kind="ExternalInput").ap()      # (half, cc, f)-major
    wq0_d = nc.dram_tensor("w_q0", [128, CCH * 128], SD,
                           kind="ExternalInput").ap()     # hp0 of wq
    wk0_d = nc.dram_tensor("w_k0", [128, CCH * 128], SD,
                           kind="ExternalInput").ap()     # hp0 of wk
    wproj_d = nc.dram_tensor("w_proj", [128, CCH * c], SD,
                             kind="ExternalInput").ap()   # (cc, f)-major
    # y ships as bf16 (halves the output-DMA drain; host upconverts).
    # Costs ~0.3% extra rounding on y only - budget is 2e-2, we run 5.4e-3.
    out_d = nc.dram_tensor("out", [t, c], SD, kind="ExternalOutput").ap()

    Exp = mybir.ActivationFunctionType.Exp
    Copy = mybir.ActivationFunctionType.Copy

    units = [(b, hp) for b in range(bl) for hp in range(NHP)]
    NU = len(units)     # 12

    with tile.TileContext(nc) as tc, ExitStack() as ctx:
        consts = ctx.enter_context(tc.tile_pool(name="consts", bufs=1))
        xp = ctx.enter_context(tc.tile_pool(name="xp", bufs=2))
        qkp = ctx.enter_context(tc.tile_pool(name="qkp", bufs=3))
        vp = ctx.enter_context(tc.tile_pool(name="vp", bufs=2))
        ep = ctx.enter_context(tc.tile_pool(name="ep", bufs=9))
        aop = ctx.enter_context(tc.tile_pool(name="aop", bufs=2))
        smp = ctx.enter_context(tc.tile_pool(name="smp", bufs=1))
        yp = ctx.enter_context(tc.tile_pool(name="yp", bufs=8))
        ps_s = ctx.enter_context(tc.tile_pool(name="ps_s", bufs=2, space="PSUM"))
        ps_f = ctx.enter_context(tc.tile_pool(name="ps_f", bufs=4, space="PSUM"))

        # ---------------- DMA prologue --------------------------------------
        # All DMAs are straight partition-major copies. Per-queue DMA rate is
        # only ~105 GB/s, so the ramp-critical stream (hp0 weights + x(b0))
        # is spread over all three DMA queues in chase order, small pieces
        # first:
        #   sync:   x0-cc0 | x0-cc1 | wqA(hp1-2) | x1g0 | x1g1
        #   gpsimd: wq0 | x0-cc2,3 | wvA(h0) | wkA(hp1-2) | wkB | x1g2
        #   scalar: wk0 | x0-cc4,5 | wvB(h1) | wqB(hp3-5) | wproj
        wq0_sb = consts.tile([128, CCH, 128], SD, tag="wq0")
        wk0_sb = consts.tile([128, CCH, 128], SD, tag="wk0")
        wq_hp0 = [wq0_sb[:, cc, :] for cc in range(CCH)]
        wk_hp0 = [wk0_sb[:, cc, :] for cc in range(CCH)]
        xg = [[xp.tile([128, 2, n], SD, tag=f"xg{g}", name=f"xg_b{b}g{g}")
               for g in range(NXG)] for b in range(bl)]
        xT_all = [[[xg[b][cc // 2][:, cc % 2, xh * 512:(xh + 1) * 512]
                    for xh in range(NXH)] for cc in range(CCH)]
                  for b in range(bl)]
        wqt = consts.tile([128, NHP, CCH, 128], SD, tag="wqt")
        wkt = consts.tile([128, NHP, CCH, 128], SD, tag="wkt")
        wvt = consts.tile([128, 2, CCH, PH], SD, tag="wvt")
        wpt = consts.tile([128, CCH, c], SD, tag="wpt")
        wproj_sb = [wpt[:, cc, :] for cc in range(CCH)]

        def wq_ap(cc, hp):
            return wqt[:, hp, cc, :]

        def wk_ap(cc, hp):
            return wkt[:, hp, cc, :]

        def wv_ap(cc, half):
            return wvt[:, half, cc, :]

        def r3(dram_ap, j):
            return dram_ap.rearrange("p (j f) -> p j f", j=j)

        def x_dma(q, b, cc0, cc1):
            g, j0 = cc0 // 2, cc0 % 2
            q.dma_start(
                out=xg[b][g][:, j0:j0 + (cc1 - cc0)],
                in_=r3(xT_d[(b * NXG + g) * 128:(b * NXG + g + 1) * 128,
                            j0 * n:(j0 + (cc1 - cc0)) * n], cc1 - cc0))

        nc.gpsimd.dma_start(out=wq0_sb, in_=r3(wq0_d[:, :], CCH))
        nc.scalar.dma_start(out=wk0_sb, in_=r3(wk0_d[:, :], CCH))
        x_dma(nc.sync, 0, 0, 1)
        x_dma(nc.sync, 0, 1, 2)
        x_dma(nc.sync, 0, 2, 3)
        x_dma(nc.gpsimd, 0, 3, 4)
        x_dma(nc.gpsimd, 0, 4, 5)
        x_dma(nc.scalar, 0, 5, 6)

        def w_piece(q, dst, src, h0, h1):
            q.dma_start(
                out=dst[:, h0:h1],
                in_=src[:, h0 * CCH * 128:h1 * CCH * 128].rearrange(
                    "p (hp cc f) -> p hp cc f", hp=h1 - h0, cc=CCH))

        # wv in cc-range pieces so v-groups chase the stream (accumulation
        # starts at cc0 long before the tail chunks land)
        for vh, q in ((0, nc.gpsimd), (1, nc.scalar)):
            for c0, c1 in ((0, 3), (3, CCH)):
                q.dma_start(
                    out=wvt[:, vh, c0:c1],
                    in_=wv_d[:, (vh * CCH + c0) * PH:(vh * CCH + c1) * PH]
                    .rearrange("p (cc f) -> p cc f", cc=c1 - c0))
        w_piece(nc.sync, wqt, wq_d, 1, 3)
        w_piece(nc.gpsimd, wkt, wk_d, 1, 3)
        w_piece(nc.gpsimd, wkt, wk_d, 3, 6)
        w_piece(nc.scalar, wqt, wq_d, 3, 6)
        for g, q in ((0, nc.sync), (1, nc.sync), (2, nc.gpsimd)):
            nc_q = q
            nc_q.dma_start(
                out=xg[1][g],
                in_=r3(xT_d[(NXG + g) * 128:(NXG + g + 1) * 128, :], 2))
        nc.scalar.dma_start(out=wpt, in_=r3(wproj_d[:, :], CCH))

        # ---------------- building-block emitters --------------------------
        qt_all = {}   # (b, hp) -> [128, n] q^T tile (2 heads stacked)
        kt_all = {}
        v_all = [[None] * NTT for _ in range(bl)]
        e_all = {}    # (b, hp, kt, head) -> E tile
        u_ps = {}     # (b, hp) -> [head][qn] psum accumulators
        ao_all = {}   # (b, hp) -> [128, n] normalized attention output^T
        y_tiles = {}  # (b, tt) -> ([128, c] tile, halves-finished count)

        def emit_qk_group(b, hp, dst, qn):
            """Project one 512-token slice of q^T (dst=0) or k^T (dst=1)."""
            key = (b, hp)
            store = qt_all if dst == 0 else kt_all
            if key not in store:
                store[key] = qkp.tile([128, n], SD, tag=f"qk{dst}",
                                      name=f"{'qk'[dst]}t_b{b}hp{hp}")
            ps = ps_f.tile([128, 512], FP32, tag="u",
                           name=f"qkps_b{b}hp{hp}d{dst}q{qn}")
            # unit 0 accumulates in DMA-arrival order (x pieces land
            # out-of-order across the three queues); psum accumulation
            # is order-independent
            ccs = [0, 5, 3, 4, 1, 2] if (b, hp) == (0, 0) \
                else list(range(CCH))
            for j, cc in enumerate(ccs):
                if hp == 0:
                    w_ap = (wq_hp0 if dst == 0 else wk_hp0)[cc]
                else:
                    w_ap = (wq_ap if dst == 0 else wk_ap)(cc, hp)
                nc.tensor.matmul(
                    ps,
                    lhsT=mm(w_ap),
                    rhs=mm(xT_all[b][cc][qn]),
                    start=(j == 0), stop=(j == CCH - 1))
            with tc.high_priority(offset=300):
                nc.vector.tensor_copy(
                    store[key][:, qn * 512:(qn + 1) * 512], ps)

        def emit_v_group(b, tt, half):
            """One [128-token, 384-channel] slice of v' (+ones cols)."""
            if half == 0:
                vt = vp.tile([128, h * VW], SD, tag=f"v{tt}",
                             name=f"v_b{b}t{tt}")
                ones_view = vt[:, :].rearrange(
                    "p (hh w) -> p hh w", hh=h)[:, :, hd:hd + 1]
                nc.gpsimd.memset(ones_view, 1.0)
                v_all[b][tt] = vt
            vt = v_all[b][tt]
            ps = ps_f.tile([128, PH], FP32, tag="u",
                           name=f"vps_b{b}t{tt}f{half}")
            xh, tl = tt // 4, tt % 4
            for cc in range(CCH):
                nc.tensor.matmul(
                    ps,
                    lhsT=mm(xT_all[b][cc][xh][:, tl * 128:(tl + 1) * 128]),
                    rhs=mm(wv_ap(cc, half)),
                    start=(cc == 0), stop=(cc == CCH - 1))
            nheads = PH // hd
            dst = vt[:, half * nheads * VW:(half + 1) * nheads * VW].rearrange(
                "p (hh w) -> p hh w", hh=nheads)[:, :, 0:hd]
            srcv = ps[:].rearrange("p (hh w) -> p hh w", hh=nheads)
            with tc.high_priority(offset=300):
                nc.vector.tensor_copy(dst, srcv)

        def get_y_tile(b, tt):
            if (b, tt) not in y_tiles:
                y_tiles[(b, tt)] = [yp.tile([128, c], SD, tag="y",
                                            name=f"y_b{b}t{tt}"), 0]
            return y_tiles[(b, tt)]

        tail_q = [0]

        def evac_y_half(b, tt, half, ps, evac, bump=True):
            """Copy/accumulate proj psum into the y tile; DMA when complete."""
            ent = get_y_tile(b, tt)
            yt = ent[0]
            dstv = yt[:, half * PH:(half + 1) * PH]
            if evac == "scalar":
                nc.scalar.activation(dstv, ps, Copy)
            elif evac == "add":
                with tc.high_priority(offset=300):
                    nc.vector.tensor_add(dstv, ps, dstv)
            else:
                with tc.high_priority(offset=300):
                    nc.vector.tensor_copy(dstv, ps)
            if bump:
                ent[1] += 1
                if ent[1] == 2:
                    if b == 0:
                        q = nc.sync if tt % 2 == 0 else nc.gpsimd
                    else:
                        qs = [nc.sync, nc.gpsimd, nc.scalar]
                        q = qs[tail_q[0] % 3]
                        tail_q[0] += 1
                    q.dma_start(
                        out=out_d[b * n + tt * 128:b * n + (tt + 1) * 128, :],
                        in_=yt)
                    del y_tiles[(b, tt)]

        def emit_proj_group(b, tt, half, evac="vector"):
            """One [128-token, 384-channel] output-projection slice."""
            ps = ps_f.tile([128, PH], FP32, tag="u",
                           name=f"yps_b{b}t{tt}f{half}")
            for cc in range(CCH):
                nc.tensor.matmul(
                    ps,
                    lhsT=mm(ao_all[(b, cc)][:, tt * 128:(tt + 1) * 128]),
                    rhs=mm(wproj_sb[cc][:, half * PH:(half + 1) * PH]),
                    start=(cc == 0), stop=(cc == CCH - 1))
            evac_y_half(b, tt, half, ps, evac)

        s_done = set()

        def emit_S(b, hp, kt):
            """S^T matmuls + exp for both heads of one 128-key tile."""
            if (b, hp, kt) in s_done:
                return
            s_done.add((b, hp, kt))
            qb = qt_all[(b, hp)]
            kb = kt_all[(b, hp)]
            for head in range(2):
                p0 = head * 64
                sps = ps_s.tile([128, n], FP32, tag="s",
                                name=f"s_b{b}hp{hp}k{kt}h{head}")
                for qn in range(NQ):
                    nc.tensor.matmul(
                        sps[:, qn * 512:(qn + 1) * 512],
                        lhsT=mm(kb[p0:p0 + 64, kt * 128:(kt + 1) * 128]),
                        rhs=mm(qb[p0:p0 + 64, qn * 512:(qn + 1) * 512]),
                        start=True, stop=True)
                et = ep.tile([128, n], SD, tag=f"e{head}",
                             name=f"e_b{b}hp{hp}k{kt}h{head}")
                nc.scalar.activation(et, sps, Exp, scale=scale)
                e_all[(b, hp, kt, head)] = et

        def emit_U_chunk(b, hp, kts):
            """U matmuls for key-tiles `kts`; their E tiles are all ready."""
            if (b, hp) not in u_ps:
                u_ps[(b, hp)] = [[ps_f.tile([VW, 512], FP32, tag="u",
                                            name=f"u_b{b}hp{hp}h{hh}q{qn}")
                                  for qn in range(NQ)] for hh in range(2)]
            ups = u_ps[(b, hp)]
            for kt in kts:
                for head in range(2):
                    hh = 2 * hp + head
                    et = e_all.pop((b, hp, kt, head))
                    for qn in range(NQ):
                        nc.tensor.matmul(
                            ups[head][qn],
                            lhsT=mm(v_all[b][kt][:, hh * VW:hh * VW + VW]),
                            rhs=mm(et[:, qn * 512:(qn + 1) * 512]),
                            start=(kt == 0), stop=(kt == NKT - 1))

        def emit_norm_evac(b, hp):
            """Copy both heads' U psums to SBUF (frees the flex ring)."""
            ups = u_ps.pop((b, hp))
            usb = {}
            for head in (1, 0):
                usb[head] = smp.tile([VW, n], FP32, tag=f"usb{head}",
                                     name=f"usb_b{b}hp{hp}h{head}")
            # qn-major: the last unit's split-norm first half-chain
            # (tokens 0-511) starts after the first two copies
            for qn in range(NQ):
                for head in (1, 0):
                    with tc.high_priority(offset=300):
                        nc.vector.tensor_copy(
                            usb[head][:, qn * 512:(qn + 1) * 512],
                            ups[head][qn])
            return usb

        def norm_cols(b, hp, usb, ao, c0, c1):
            """Divide one column range by Z, fill that range of aoT.

            Z -> partition 0 (DMA), broadcast to 64 partitions (gpsimd),
            reciprocal on the broadcast tile (base-partition!=0 sources
            mis-execute on hw for both the DVE op and the broadcast).
            """
            w = c1 - c0
            for head in (1, 0):
                ut = usb[head]
                z1 = smp.tile([1, n], FP32, tag=f"z1{head}", bufs=1,
                              name=f"z1_b{b}hp{hp}h{head}c{c0}")
                nc.sync.dma_start(out=z1[:, 0:w], in_=ut[hd:hd + 1, c0:c1])
                rb = smp.tile([64, n], FP32, tag=f"rb{head}",
                              name=f"rb_b{b}hp{hp}h{head}c{c0}")
                nc.gpsimd.partition_broadcast(rb[:, 0:w], z1[:, 0:w])
                nc.vector.reciprocal_approx_fast(rb[:, 0:w], rb[:, 0:w])
                if head == 0:
                    nc.vector.tensor_mul(ao[0:64, c0:c1], ut[0:hd, c0:c1],
                                         rb[:, 0:w])
                else:
                    sc = smp.tile([64, n], SD, tag="sc",
                                  name=f"sc_b{b}hp{hp}c{c0}")
                    nc.vector.tensor_mul(sc[:, 0:w], ut[0:hd, c0:c1],
                                         rb[:, 0:w])
                    nc.sync.dma_start(out=ao[64:128, c0:c1], in_=sc[:, 0:w])

        def alloc_ao(b, hp):
            ao = aop.tile([128, n], SD, tag=f"ao{hp}", name=f"ao_b{b}hp{hp}")
            ao_all[(b, hp)] = ao
            return ao

        def emit_norm_math(b, hp, usb):
            norm_cols(b, hp, usb, alloc_ao(b, hp), 0, n)

        def emit_norm(b, hp):
            emit_norm_math(b, hp, emit_norm_evac(b, hp))

        # proj(b1) groups: cc0..4 partials land in the y tiles early, a
        # single cc5 "final" + in-place add completes them after the last
        # norm. rest = the 12 non-warm groups.
        rest = [(tt, half) for tt in range(2, NTT) for half in range(2)]
        partial_done = set()

        def proj_mms(tt, half, ps, ccs, start, stop):
            for cc in ccs:
                nc.tensor.matmul(
                    ps,
                    lhsT=mm(ao_all[(1, cc)][:, tt * 128:(tt + 1) * 128]),
                    rhs=mm(wproj_sb[cc][:, half * PH:(half + 1) * PH]),
                    start=start and cc == ccs[0],
                    stop=stop and cc == ccs[-1])

        def emit_partial_rest(g):
            if g in partial_done:
                return
            partial_done.add(g)
            tt, half = rest[g]
            ps = ps_f.tile([128, PH], FP32, tag="u", name=f"ypart{g}")
            proj_mms(tt, half, ps, range(CCH - 1), True, True)
            evac_y_half(1, tt, half, ps,
                        "scalar" if g % 2 == 0 else "vector", bump=False)

        # ---------------- filler schedule ----------------------------------
        # per-unit list of thunks run between S groups of that unit
        fillers = [[] for _ in range(NU)]

        def add_qk_fillers(i, b, hp):
            for qn in range(NQ):
                for dst in range(2):
                    fillers[i].append(
                        lambda b=b, hp=hp, dst=dst, qn=qn:
                        emit_qk_group(b, hp, dst, qn))

        # unit 0: v(b0) fully (half0 first: its wv piece lands first) + qk(u1)
        for half in range(2):
            for tt in range(NTT):
                fillers[0].append(
                    lambda tt=tt, half=half: emit_v_group(0, tt, half))
        add_qk_fillers(0, *units[1])
        # units 1..4: qk(next) + v(b1) spread 4 per unit
        for i in range(1, 5):
            add_qk_fillers(i, *units[i + 1])
        vq = [(tt, half) for tt in range(NTT) for half in range(2)]
        for j, (tt, half) in enumerate(vq):
            fillers[1 + j // 4].append(
                lambda tt=tt, half=half: emit_v_group(1, tt, half))
        # units 5..10: qk(next)
        for i in range(5, 11):
            add_qk_fillers(i, *units[i + 1])
        # units 7..10: proj(b0)  (all ao(b0) ready after norm(u5) in unit 6;
        # unit 11 keeps its flex psum free for in-unit U accumulation)
        pq = [(tt, half) for tt in range(NTT) for half in range(2)]
        for j, (tt, half) in enumerate(pq):
            fillers[7 + j % 4].append(
                lambda tt=tt, half=half: emit_proj_group(0, tt, half))

        # ---------------- main schedule ------------------------------------
        # prologue: qk(unit0), qn-major to chase the x DMA stream
        b0, hp0 = units[0]
        for qn in range(NQ):
            for dst in range(2):
                emit_qk_group(b0, hp0, dst, qn)

        for i, (b, hp) in enumerate(units):
            fl = list(fillers[i])
            prev = units[i - 1] if i > 0 else None
            # S(k0) first so the Scalar engine stays fed across the boundary;
            # previous unit's U matmuls run in chunks between S groups so
            # exp never starves and every U operand is long since ready.
            emit_S(b, hp, 0)
            start_kt = 1
            if prev is not None:
                for j, (k0, k1) in enumerate(((0, 2), (2, 4), (4, 6), (6, 8))):
                    emit_U_chunk(prev[0], prev[1], range(k0, k1))
                    if j < 3:
                        emit_S(b, hp, j + 1)
                emit_norm(*prev)
                start_kt = 4
            # spread fillers across the remaining kt slots; the last unit
            # instead runs its own U matmuls in-unit at lag 4
            nslots = NKT - start_kt
            tot = len(fl)
            for kt in range(start_kt, NKT):
                emit_S(b, hp, kt)
                if i == NU - 1 and kt >= 4:
                    emit_U_chunk(b, hp, [kt - 4])
                j = kt - start_kt
                k = (tot * (j + 1)) // nslots - (tot * j) // nslots
                for _ in range(k):
                    if fl:
                        fl.pop(0)()
            # pull the last unit's first S/exp group into this unit's
            # Scalar-engine slack: exp(u11,k7) gates the whole tail chain
            if i == NU - 2:
                emit_S(*units[NU - 1], 0)

        # ---------------- epilogue -----------------------------------------
        # Last unit's U(k4..7); 4 warm groups (tt0-1) keep their cc0..4
        # partials IN the freed S-slot banks (final = one more accumulating
        # matmul + plain copy evac), the other 12 groups' partials rotate
        # the flex ring and land in the y tiles (in-place add finals), all
        # while the final norm chain drains. Whole-row y DMAs on 3 queues.
        pb, php = units[-1]
        emit_U_chunk(pb, php, range(4, NKT))

        warm = [(0, 0), (0, 1), (1, 0), (1, 1)]
        s_carve = [ps_s.tile([128, n], FP32, tag="s", name=f"scarve{j}")
                   for j in range(2)]
        warm_ps = {}
        for g, (tt, half) in enumerate(warm):
            ps = s_carve[g // 2][:, (g % 2) * 512:(g % 2) * 512 + PH]
            warm_ps[(tt, half)] = ps
            proj_mms(tt, half, ps, range(CCH - 1), True, False)
        usb_last = emit_norm_evac(pb, php)
        for g in range(len(rest)):
            emit_partial_rest(g)
        # norm in qn halves: finals for tt0-3 (tokens 0-511) fire after the
        # first half-chain, their DMAs overlapping the second half
        ao_last = alloc_ao(pb, php)
        cc5 = [CCH - 1]

        def finals(tts):
            for tt in tts:
                for half in range(2):
                    if (tt, half) in warm_ps:
                        ps = warm_ps[(tt, half)]
                        proj_mms(tt, half, ps, cc5, False, True)
                        evac_y_half(1, tt, half, ps, "scalar")
                    else:
                        ps = ps_f.tile([128, PH], FP32, tag="u",
                                       name=f"yfin{tt}_{half}")
                        proj_mms(tt, half, ps, cc5, True, True)
                        evac_y_half(1, tt, half, ps, "add")

        norm_cols(pb, php, usb_last, ao_last, 0, 512)
        finals(range(0, 4))
        norm_cols(pb, php, usb_last, ao_last, 512, n)
        finals(range(4, NTT))

    nc.compile()
    return nc


_NC_CACHE = {}


def _get_nc(compute=COMPUTE):
    if compute not in _NC_CACHE:
        _NC_CACHE[compute] = build_attention_nc(compute)
    return _NC_CACHE[compute]


def make_in_maps(x, W_qkv, W_proj, b_proj, compute=None):
    compute = compute or COMPUTE
    if compute == "bf16":
        import ml_dtypes
        sd = ml_dtypes.bfloat16
    else:
        sd = np.float32
    x = np.asarray(x, dtype=np.float32)
    W_qkv = np.asarray(W_qkv, dtype=np.float32)
    CCH, NHP, PH, NXG = C // 128, H // 2, C // 2, C // 256

    def pack(w, inner):
        # [C, X] -> partition-major [128, (outer..., inner)] image
        return np.ascontiguousarray(
            w.reshape(CCH, 128, -1, inner).transpose(1, 2, 0, 3)
            .reshape(128, -1)).astype(sd)

    wq = pack(W_qkv[:, 0:C], 128)              # (hp, cc, 128)
    wk = pack(W_qkv[:, C:2 * C], 128)
    wv = pack(W_qkv[:, 2 * C:3 * C], PH)       # (half, cc, PH)
    wp = pack(np.asarray(W_proj, dtype=np.float32), C)  # (cc, C)
    wq0 = np.ascontiguousarray(
        W_qkv[:, 0:128].reshape(CCH, 128, 128).transpose(1, 0, 2)
        .reshape(128, -1)).astype(sd)
    wk0 = np.ascontiguousarray(
        W_qkv[:, C:C + 128].reshape(CCH, 128, 128).transpose(1, 0, 2)
        .reshape(128, -1)).astype(sd)
    in_maps = []
    for i in range(NCORES):
        shard = x[i * BL:(i + 1) * BL]                      # [BL, N, C]
        # (b, g, p, j, f) image: rows (b, g, p), cols (j, f)
        xT = np.ascontiguousarray(
            shard.transpose(0, 2, 1).reshape(BL, NXG, 2, 128, N)
            .transpose(0, 1, 3, 2, 4).reshape(BL * NXG * 128, 2 * N)
        ).astype(sd)
        in_maps.append({"xT": xT, "w_q": wq, "w_k": wk, "w_v": wv,
                        "w_q0": wq0, "w_k0": wk0, "w_proj": wp})
    return in_maps


def kernel(x, W_qkv, W_proj, b_proj):
    from concourse.bass_utils import run_bass_kernel_spmd

    nc = _get_nc()
    in_maps = make_in_maps(x, W_qkv, W_proj, b_proj)
    res = run_bass_kernel_spmd(nc, in_maps, core_ids=list(range(NCORES)))
    outs = [res.results[i]["out"].reshape(BL, N, C) for i in range(NCORES)]
    y = np.concatenate(outs, axis=0).astype(np.float32)
    return y + np.asarray(b_proj, dtype=np.float32)


if __name__ == "__main__":
    nc = build_attention_nc()
    print("built ok")
